# revision 28
# baseline (speedup 1.0000x reference)
"""Bass/Trainium2 kernel for nn_Head_13030930776875.

out = 0.7*softmax(causal(x@Wq @ (x@Wk)^T / sqrt(d))) @ (x@Wv)
    + 0.3*rownorm(causal(exp(-|y_i - y_j|^2 / (2d)))) @ (x@Wv),  y = (x@Wk)@L_grav

Sharding: 8 cores = 4 samples x 2 halves. Each half owns two 512-row query
groups chosen so causal (triangular) work balances: half0 -> {G0, G3},
half1 -> {G1, G2}.

Permuted key layout: the host ships x^T with its 512-column groups permuted
per half (h0: [G0,G1,G3,G2], h1: [G1,G0,G2,G3]) so that SBUF group 0 is the
pos0 query group and group 2 is the pos1 query group FOR EVERY CORE. The
program is SPMD-uniform; causality over the permuted key order is entirely
data-driven via host-baked threshold masks. This kills the separate x-query
gather (2MB DMA) and the Wy projection: qT projects from fixed slices of
x^T, and the grav query-side y is just a slice of yT.

On-device layout: everything transposed (d on partitions). Scores are
computed as s^T tiles [k,q] so that (a) A^T slices feed the A@v matmul
directly as the stationary operand (no transposes anywhere), and (b) the
causal row-sums come free via a ones-column appended to v.

Score exps are biasless and batched: two 512-wide score matmuls share one
[128,1024] PSUM tile and a single exp ACT. The grav kernel factorizes as
exp(gram/128)*exp(-sq_k/256); the per-key factor ek is pre-multiplied into
the grav masks (mkg) during idle DVE time. Fully-invalid chunks get an
all-zero mask, so no per-chunk exp bias is needed anywhere.
"""

import math
import os

import numpy as np

B, N, D_MODEL, D_HEAD = 4, 2048, 1024, 128
OMEGA_LANG, OMEGA_GRAV = 0.7, 0.3
SC_LANG = 1.0 / math.sqrt(D_HEAD)
SC_GRAV = 1.0 / D_HEAD
NBLK = N // 128            # 16 k-chunks of 128
NCH = (8, 16)              # chunks per position (pos0 group, pos1 group)
NSLOT = 16                 # mask slots: pos0 loop 0-7, pos1 loop 8-15
# chunk processing order per position (band group last so the A@v prefix
# structure nkb = 5+j / 13+j covers exactly the causally-needed chunks)
P0 = [4, 5, 6, 7, 0, 1, 2, 3]
P1 = [0, 1, 2, 3, 4, 5, 6, 7, 12, 13, 14, 15, 8, 9, 10, 11]
# original group id per SBUF group slot, per half
GROUPS = [[0, 1, 3, 2], [1, 0, 2, 3]]

_CACHE = {}


def _build_nc():
    import concourse.bacc as bacc
    import concourse.mybir as mybir
    import concourse.tile as tile
    import concourse.bass as bass

    dt = mybir.dt
    F16, F32 = dt.float16, dt.float32
    AF = mybir.ActivationFunctionType
    OP = mybir.AluOpType

    nc = bacc.Bacc()

    xT = nc.declare_dram_parameter("xT", [D_MODEL, N], F16, isOutput=False)
    wq = nc.declare_dram_parameter("wq", [128, 8 * 128], F16, isOutput=False)
    wk = nc.declare_dram_parameter("wk", [128, 8 * 128], F16, isOutput=False)
    wv = nc.declare_dram_parameter("wv", [128, 8 * 128], F16, isOutput=False)
    lg = nc.declare_dram_parameter("lg", [128, 128], F16, isOutput=False)
    mk = nc.declare_dram_parameter("mk", [128, NSLOT * 512], F16, isOutput=False)
    ident = nc.declare_dram_parameter("ident", [128, 128], F16, isOutput=False)
    out_d = nc.declare_dram_parameter("out", [N // 2, 128], F32, isOutput=True)

    with tile.TileContext(nc) as tc:
        with (
            tc.tile_pool(name="big", bufs=1) as big,
            tc.tile_pool(name="xtp", bufs=8) as xtp,
            tc.tile_pool(name="ap0", bufs=1) as ap0,
            tc.tile_pool(name="ap1", bufs=1) as ap1,
            tc.tile_pool(name="small", bufs=4) as small,
            tc.tile_pool(name="outp", bufs=4) as outp,
            tc.tile_pool(name="score", bufs=2, space="PSUM") as score,
            tc.tile_pool(name="pp", bufs=2, space="PSUM") as pp,
            tc.tile_pool(name="av", bufs=2, space="PSUM") as av,
        ):
            # ---- small inputs on the scalar DGE queue (parallel to x) ----
            wq_s = big.tile([128, 8, 128], F16, tag="wq")
            wk_s = big.tile([128, 8, 128], F16, tag="wk")
            wv_s = big.tile([128, 8, 128], F16, tag="wv")
            nc.sync.dma_start(wk_s[:], wk[:].rearrange("p (c d) -> p c d", c=8))
            lg_s = big.tile([128, 128], F16, tag="lg")
            nc.sync.dma_start(lg_s[:], lg[:])
            id_s = big.tile([128, 128], F16, tag="id")
            nc.sync.dma_start(id_s[:], ident[:])

            # ---- x^T in 1024-col halves (2KB DMA lines), need-ordered ----
            xt = [xtp.tile([128, N], F16, tag="xt", name=f"xt{c}")
                  for c in range(8)]
            for c in range(8):
                nc.sync.dma_start(xt[c][:, 0:1024], xT[c * 128:(c + 1) * 128, 0:1024])
            nc.sync.dma_start(wq_s[:], wq[:].rearrange("p (c d) -> p c d", c=8))
            nc.sync.dma_start(wv_s[:], wv[:].rearrange("p (c d) -> p c d", c=8))
            # host-baked causal masks (per-half data): pos0 slots first
            mk_s = big.tile([128, NSLOT * 512], F16, tag="mk")
            nc.sync.dma_start(mk_s[:, 0:4096], mk[:, 0:4096])
            for c in range(8):
                nc.sync.dma_start(xt[c][:, 1024:2048],
                                  xT[c * 128:(c + 1) * 128, 1024:2048])
            nc.sync.dma_start(mk_s[:, 4096:8192], mk[:, 4096:8192])

            # ---- persistent intermediates ----
            kT = big.tile([128, N], F16, tag="kT")
            qT = big.tile([128, N // 2], F16, tag="qT")
            yT = big.tile([128, N], F16, tag="yT")
            sqn = big.tile([128, NBLK], F32, tag="sqn")
            ek_s = big.tile([128, NBLK], F32, tag="ek")
            mkg = big.tile([128, NSLOT * 512], F16, tag="mkg")
            vaug = big.tile([128, NBLK, 132], F16, tag="vaug")
            nc.vector.memset(vaug[:, :, 128:129], 1.0 / OMEGA_LANG)
            nc.vector.memset(vaug[:, :, 129:130], 1.0 / OMEGA_GRAV)

            def proj_group(dst, dcol, w_sb, g):
                cols = slice(g * 512, (g + 1) * 512)
                ps = pp.tile([128, 512], F32, tag="pp")
                for c in range(8):
                    nc.tensor.matmul(ps[:], w_sb[:, c, :], xt[c][:, cols],
                                     start=(c == 0), stop=(c == 7))
                nc.vector.tensor_copy(dst[:, dcol * 512:(dcol + 1) * 512], ps[:])

            def yt_group(g):
                cols = slice(g * 512, (g + 1) * 512)
                ps = pp.tile([128, 512], F32, tag="pp")
                nc.tensor.matmul(ps[:], lg_s[:], kT[:, cols])
                nc.vector.tensor_copy(yT[:, cols], ps[:])

            def sqn_quad(q4):
                # y chunks for 4 key blocks -> [128,4,128] psum; squared on
                # ACT ((y/16)^2 = y^2/256), then one DVE reduce -> sq/256.
                ps = pp.tile([128, 4, 128], F32, tag="pp")
                for i in range(4):
                    kb = q4 * 4 + i
                    nc.tensor.matmul(ps[:, i, :], kT[:, kb * 128:(kb + 1) * 128],
                                     lg_s[:])
                scr = small.tile([128, 4, 128], F32, tag="scr")
                nc.scalar.activation(scr[:], ps[:], AF.Square, scale=0.0625)
                nc.vector.tensor_reduce(sqn[:, q4 * 4:(q4 + 1) * 4], scr[:],
                                        mybir.AxisListType.X, OP.add)

            vT = big.tile([128, N], F16, tag="vT")

            def vt_group(g):
                ps = pp.tile([128, 512], F32, tag="pp")
                cols = slice(g * 512, (g + 1) * 512)
                for c in range(8):
                    nc.tensor.matmul(ps[:], wv_s[:, c, :], xt[c][:, cols],
                                     start=(c == 0), stop=(c == 7))
                nc.vector.tensor_copy(vT[:, cols], ps[:])

            def vaug_chunk(kb):
                # v chunk [k,d] = PE-transpose of vT columns (f16 psum)
                ps = av.tile([128, 128], F16, tag="av")
                nc.tensor.transpose(ps[:], vT[:, kb * 128:(kb + 1) * 128],
                                    id_s[:])
                nc.vector.tensor_copy(vaug[:, kb, 0:128], ps[:])

            # PE warmup woven into the first kT projection group: 3 dummy
            # matmuls per x chunk, each consuming only already-arrived data,
            # so the PE tracks DMA arrival and the HAM clock-gate opens
            # (4/8 -> 8/8) by the time the dense stream begins.
            def kt_g0_with_warmup():
                warm = pp.tile([128, 512], F32, tag="pp")
                ps = pp.tile([128, 512], F32, tag="pp")
                for c in range(8):
                    for i in range(3):
                        nc.tensor.matmul(warm[:], xt[c][:, 0:128],
                                         xt[c][:, 0:512],
                                         start=(c == 0 and i == 0),
                                         stop=(c == 7 and i == 2))
                    nc.tensor.matmul(ps[:], wk_s[:, c, :], xt[c][:, 0:512],
                                     start=(c == 0), stop=(c == 7))
                nc.vector.tensor_copy(kT[:, 0:512], ps[:])

            def score_pair(dst, lhs, slot, plist, qrhs, scale):
                # two 512-wide score matmuls (chunks plist[slot], plist[slot+1])
                # -> one [128,1024] psum tile, one biasless exp ACT.
                ca, cb = plist[slot], plist[slot + 1]
                ps = score.tile([128, 1024], F32, tag="sc")
                nc.tensor.matmul(ps[:, 0:512], lhs[:, ca * 128:(ca + 1) * 128], qrhs)
                nc.tensor.matmul(ps[:, 512:1024], lhs[:, cb * 128:(cb + 1) * 128],
                                 qrhs)
                nc.scalar.activation(dst[:, slot * 512:(slot + 2) * 512], ps[:],
                                     AF.Exp, scale=scale)

            # ================= pos0 prologue =================
            kt_g0_with_warmup()
            proj_group(kT, 1, wk_s, 1)
            proj_group(qT, 0, wq_s, 0)
            q0 = qT[:, 0:512]
            yq0 = yT[:, 0:512]

            alang = [None, None]
            agrav = [None, None]
            alang[0] = ap0.tile([128, NCH[0] * 512], F16, tag="al0", name="al0")
            agrav[0] = ap0.tile([128, NCH[0] * 512], F16, tag="ag0", name="ag0")

            # lang scores pos0 interleaved with remaining prologue PE work
            score_pair(alang[0], kT, 0, P0, q0, SC_LANG)
            yt_group(0)
            score_pair(alang[0], kT, 2, P0, q0, SC_LANG)
            yt_group(1)
            score_pair(alang[0], kT, 4, P0, q0, SC_LANG)
            sqn_quad(0)
            score_pair(alang[0], kT, 6, P0, q0, SC_LANG)
            sqn_quad(1)
            nc.scalar.activation(ek_s[:, 0:8], sqn[:, 0:8], AF.Exp, scale=-1.0)
            for s in range(8):
                mcols = slice(s * 512, (s + 1) * 512)
                nc.vector.tensor_scalar(mkg[:, mcols], mk_s[:, mcols],
                                        ek_s[:, P0[s]:P0[s] + 1], None, OP.mult)

            # grav scores pos0 interleaved with vaug chunks
            vt_group(0)
            vt_group(1)
            for p in range(4):
                score_pair(agrav[0], yT, 2 * p, P0, yq0, SC_GRAV)
                vaug_chunk(2 * p)
                vaug_chunk(2 * p + 1)

            # pos0 masking: lang mask-mult, grav ek-premultiplied mask-mult
            for s in range(8):
                acols = slice(s * 512, (s + 1) * 512)
                nc.vector.tensor_tensor(alang[0][:, acols], alang[0][:, acols],
                                        mk_s[:, acols], OP.mult)
                nc.vector.tensor_tensor(agrav[0][:, acols], agrav[0][:, acols],
                                        mkg[:, acols], OP.mult)

            # ================= pos1 prologue (fills PE while pos0 exp/mask run)
            proj_group(kT, 2, wk_s, 2)
            proj_group(kT, 3, wk_s, 3)
            proj_group(qT, 1, wq_s, 2)
            q1 = qT[:, 512:1024]
            yq1 = yT[:, 1024:1536]

            # pos1 lang scores interleaved with rest of pos1 prologue
            alang[1] = ap1.tile([128, NCH[1] * 512], F16, tag="al1", name="al1")
            agrav[1] = ap1.tile([128, NCH[1] * 512], F16, tag="ag1", name="ag1")
            score_pair(alang[1], kT, 0, P1, q1, SC_LANG)
            yt_group(2)
            score_pair(alang[1], kT, 2, P1, q1, SC_LANG)
            yt_group(3)
            score_pair(alang[1], kT, 4, P1, q1, SC_LANG)
            sqn_quad(2)
            score_pair(alang[1], kT, 6, P1, q1, SC_LANG)
            sqn_quad(3)
            score_pair(alang[1], kT, 8, P1, q1, SC_LANG)
            nc.scalar.activation(ek_s[:, 8:16], sqn[:, 8:16], AF.Exp, scale=-1.0)
            for s in range(8, 16):
                mcols = slice(s * 512, (s + 1) * 512)
                nc.vector.tensor_scalar(mkg[:, mcols], mk_s[:, mcols],
                                        ek_s[:, P1[s]:P1[s] + 1], None, OP.mult)
            score_pair(alang[1], kT, 10, P1, q1, SC_LANG)
            vt_group(2)
            vaug_chunk(8)
            vaug_chunk(9)
            score_pair(alang[1], kT, 12, P1, q1, SC_LANG)
            vt_group(3)
            vaug_chunk(10)
            vaug_chunk(11)
            score_pair(alang[1], kT, 14, P1, q1, SC_LANG)
            for kb in range(12, 16):
                vaug_chunk(kb)

            def attn_j(pos, j):
                # A^T @ v_aug for one 128-row query block; rowsums ride col 128.
                nkb = (5 + j) if pos == 0 else (13 + j)
                plist = P0 if pos == 0 else P1
                pol = av.tile([128, 132], F32, tag="av")
                pog = av.tile([128, 132], F32, tag="av")
                for kb in range(nkb):
                    nc.tensor.matmul(pol[:, 0:129],
                                     alang[pos][:, kb * 512 + j * 128:kb * 512 + (j + 1) * 128],
                                     vaug[:, plist[kb], 0:129],
                                     start=(kb == 0), stop=(kb == nkb - 1))
                for kb in range(nkb):
                    nc.tensor.matmul(pog[:, 0:130],
                                     agrav[pos][:, kb * 512 + j * 128:kb * 512 + (j + 1) * 128],
                                     vaug[:, plist[kb], 0:130],
                                     start=(kb == 0), stop=(kb == nkb - 1))
                rl = small.tile([128, 1], F32, tag="rl")
                rg = small.tile([128, 1], F32, tag="rg")
                nc.vector.reciprocal(rl[:], pol[:, 128:129])
                nc.vector.reciprocal(rg[:], pog[:, 129:130])
                ob = outp.tile([128, 128], F32, tag="ob")
                ob2 = outp.tile([128, 128], F32, tag="ob2")
                nc.vector.tensor_scalar(ob[:], pol[:, 0:128], rl[:], None, OP.mult)
                nc.vector.scalar_tensor_tensor(ob2[:], pog[:, 0:128], rg[:], ob[:],
                                               OP.mult, OP.add)
                r0 = pos * 512 + j * 128
                nc.sync.dma_start(out_d[r0:r0 + 128, :], ob2[:])

            # pos0 A@v interleaved with pos1 grav scores (keeps ACT fed).
            # pos1 masking rides along per-slot: loop slots 0-7 are fully
            # valid for both halves (grav needs only the ek factor;
            # P1[0..7] == chunks 0..7); slots 8-15 get mask multiplies.
            def post1(s):
                acols = slice(s * 512, (s + 1) * 512)
                if s < 8:
                    nc.vector.tensor_scalar(agrav[1][:, acols],
                                            agrav[1][:, acols],
                                            ek_s[:, s:s + 1], None, OP.mult)
                else:
                    nc.vector.tensor_tensor(alang[1][:, acols],
                                            alang[1][:, acols],
                                            mk_s[:, acols], OP.mult)
                    nc.vector.tensor_tensor(agrav[1][:, acols],
                                            agrav[1][:, acols],
                                            mkg[:, acols], OP.mult)

            score_pair(agrav[1], yT, 0, P1, yq1, SC_GRAV)
            post1(0); post1(1)
            attn_j(0, 0)
            score_pair(agrav[1], yT, 2, P1, yq1, SC_GRAV)
            post1(2); post1(3)
            score_pair(agrav[1], yT, 4, P1, yq1, SC_GRAV)
            post1(4); post1(5)
            attn_j(0, 1)
            score_pair(agrav[1], yT, 6, P1, yq1, SC_GRAV)
            post1(6); post1(7)
            score_pair(agrav[1], yT, 8, P1, yq1, SC_GRAV)
            post1(8); post1(9)
            attn_j(0, 2)
            score_pair(agrav[1], yT, 10, P1, yq1, SC_GRAV)
            post1(10); post1(11)
            score_pair(agrav[1], yT, 12, P1, yq1, SC_GRAV)
            post1(12); post1(13)
            attn_j(0, 3)
            score_pair(agrav[1], yT, 14, P1, yq1, SC_GRAV)
            post1(14); post1(15)

            # pos1 A@v + outputs (longest chain first, shortest last)
            for j in (3, 2, 1, 0):
                attn_j(1, j)

    nc.finalize()
    return nc


def _host_inputs(x, Wq, Wk, Wv, L_grav):
    """Build the 8 per-core input maps (permuted key layout per half)."""
    f16 = np.float16
    x = np.asarray(x, np.float32)
    Wq = np.asarray(Wq, np.float32)
    Wk = np.asarray(Wk, np.float32)
    Wv = np.asarray(Wv, np.float32)
    L = np.asarray(L_grav, np.float32)

    def warr(w):  # [1024,128] -> [128, 8*128] chunk-major for lhsT slices
        return np.ascontiguousarray(
            w.reshape(8, 128, 128).transpose(1, 0, 2).reshape(128, 8 * 128)
        ).astype(f16)

    wqa, wka, wva = warr(Wq), warr(Wk), warr(Wv)
    lga = L.astype(f16)

    def half_mask(h):
        """mk [128, 16*512] f16 masks; mask slot s covers loop slot s.

        mask[k_p, q_local] = (q_orig >= key_orig), fully data-driven over
        the permuted key order. pos0 -> slots 0-7 (chunks P0), pos1 ->
        slots 8-15 (chunks P1[8..15]).
        """
        groups = GROUPS[h]
        p = np.arange(128, dtype=np.float32)[:, None]
        q = np.arange(512, dtype=np.float32)[None, :]
        mkh = np.empty((128, NSLOT, 512), np.float32)
        for s in range(NSLOT):
            if s < 8:
                c, qg = P0[s], groups[0]
            else:
                c, qg = P1[s], groups[2]
            og = groups[c // 4]
            key_orig = og * 512 + (c % 4) * 128 + p   # [128,1]
            thr = key_orig - qg * 512
            mkh[:, s, :] = (q >= thr)
        return np.ascontiguousarray(mkh.reshape(128, NSLOT * 512)).astype(f16)

    mks = [half_mask(0), half_mask(1)]
    idn = np.eye(128, dtype=np.float32).astype(f16)
    in_maps = []
    for core in range(8):
        b, h = core // 2, core % 2
        xTb = x[b].T.astype(f16)  # [1024, 2048]
        xp = np.concatenate([xTb[:, g * 512:(g + 1) * 512] for g in GROUPS[h]],
                            axis=1)
        in_maps.append({
            "xT": np.ascontiguousarray(xp),
            "wq": wqa, "wk": wka, "wv": wva, "lg": lga,
            "mk": mks[h], "ident": idn,
        })
    return in_maps


def kernel(x, Wq, Wk, Wv, L_grav):
    import concourse.bass_utils as bass_utils

    if "nc" not in _CACHE:
        _CACHE["nc"] = _build_nc()
    nc = _CACHE["nc"]
    in_maps = _host_inputs(x, Wq, Wk, Wv, L_grav)

    trace = bool(os.environ.get("BASS_KERNEL_TRACE"))
    if trace:
        bass_utils.upload_artifacts = lambda tmpdir: f"file://{tmpdir}"
    res = bass_utils.run_bass_kernel_spmd(nc, in_maps, list(range(8)), trace=trace)
    if trace:
        _CACHE["exec_time_ns"] = res.exec_time_ns
        _CACHE["mean_exec_time_ns"] = res.mean_exec_time_ns

    out = np.empty((B, N, D_HEAD), np.float32)
    for core in range(8):
        b, h = core // 2, core % 2
        r = res.results[core]["out"]
        g0, g2 = GROUPS[h][0], GROUPS[h][2]
        out[b, g0 * 512:(g0 + 1) * 512] = r[0:512]
        out[b, g2 * 512:(g2 + 1) * 512] = r[512:1024]
    return out


# revision 29
# speedup vs baseline: 1.0516x; 1.0516x over previous
"""Bass/Trainium2 kernel for nn_Head_13030930776875.

out = 0.7*softmax(causal(x@Wq @ (x@Wk)^T / sqrt(d))) @ (x@Wv)
    + 0.3*rownorm(causal(exp(-|y_i - y_j|^2 / (2d)))) @ (x@Wv),  y = (x@Wk)@L_grav

Sharding: 8 cores = 4 samples x 2 halves. Each half owns two 512-row query
groups chosen so causal (triangular) work balances: half0 -> {G0, G3},
half1 -> {G1, G2}.

Permuted key layout: the host ships x^T with its 512-column groups permuted
per half (h0: [G0,G1,G3,G2], h1: [G1,G0,G2,G3]) so that SBUF group 0 is the
pos0 query group and group 2 is the pos1 query group FOR EVERY CORE. The
program is SPMD-uniform; causality over the permuted key order is entirely
data-driven via host-baked threshold masks. This kills the separate x-query
gather (2MB DMA) and the Wy projection: qT projects from fixed slices of
x^T, and the grav query-side y is just a slice of yT.

On-device layout: everything transposed (d on partitions). Scores are
computed as s^T tiles [k,q] so that (a) A^T slices feed the A@v matmul
directly as the stationary operand (no transposes anywhere), and (b) the
causal row-sums come free via a ones-column appended to v.

Score exps are biasless and batched: two 512-wide score matmuls share one
[128,1024] PSUM tile and a single exp ACT. The grav kernel factorizes as
exp(gram/128)*exp(-sq_k/256); the per-key factor ek is pre-multiplied into
the grav masks (mkg) during idle DVE time. Fully-invalid chunks get an
all-zero mask, so no per-chunk exp bias is needed anywhere.
"""

import math
import os

import numpy as np

B, N, D_MODEL, D_HEAD = 4, 2048, 1024, 128
OMEGA_LANG, OMEGA_GRAV = 0.7, 0.3
SC_LANG = 1.0 / math.sqrt(D_HEAD)
SC_GRAV = 1.0 / D_HEAD
NBLK = N // 128            # 16 k-chunks of 128
NCH = (8, 16)              # chunks per position (pos0 group, pos1 group)
NSLOT = 16                 # mask slots: pos0 loop 0-7, pos1 loop 8-15
# chunk processing order per position (band group last so the A@v prefix
# structure nkb = 5+j / 13+j covers exactly the causally-needed chunks)
P0 = [4, 5, 6, 7, 0, 1, 2, 3]
P1 = [0, 1, 2, 3, 4, 5, 6, 7, 12, 13, 14, 15, 8, 9, 10, 11]
# original group id per SBUF group slot, per half
GROUPS = [[0, 1, 3, 2], [1, 0, 2, 3]]

_CACHE = {}


def _build_nc():
    import concourse.bacc as bacc
    import concourse.mybir as mybir
    import concourse.tile as tile
    import concourse.bass as bass

    dt = mybir.dt
    F16, F32 = dt.float16, dt.float32
    AF = mybir.ActivationFunctionType
    OP = mybir.AluOpType

    nc = bacc.Bacc()

    xT = nc.declare_dram_parameter("xT", [D_MODEL, N], F16, isOutput=False)
    wq = nc.declare_dram_parameter("wq", [128, 8 * 128], F16, isOutput=False)
    wk = nc.declare_dram_parameter("wk", [128, 8 * 128], F16, isOutput=False)
    wv = nc.declare_dram_parameter("wv", [128, 8 * 128], F16, isOutput=False)
    lg = nc.declare_dram_parameter("lg", [128, 128], F16, isOutput=False)
    mk = nc.declare_dram_parameter("mk", [128, NSLOT * 512], F16, isOutput=False)
    out_d = nc.declare_dram_parameter("out", [N // 2, 128], F32, isOutput=True)

    with tile.TileContext(nc) as tc:
        with (
            tc.tile_pool(name="big", bufs=1) as big,
            tc.tile_pool(name="xtp", bufs=8) as xtp,
            tc.tile_pool(name="ap0", bufs=1) as ap0,
            tc.tile_pool(name="ap1", bufs=1) as ap1,
            tc.tile_pool(name="small", bufs=4) as small,
            tc.tile_pool(name="outp", bufs=4) as outp,
            tc.tile_pool(name="score", bufs=2, space="PSUM") as score,
            tc.tile_pool(name="pp", bufs=2, space="PSUM") as pp,
            tc.tile_pool(name="av", bufs=2, space="PSUM") as av,
        ):
            # ---- small inputs on the scalar DGE queue (parallel to x) ----
            wq_s = big.tile([128, 8, 128], F16, tag="wq")
            wk_s = big.tile([128, 8, 128], F16, tag="wk")
            wv_s = big.tile([128, 8, 128], F16, tag="wv")
            nc.sync.dma_start(wk_s[:], wk[:].rearrange("p (c d) -> p c d", c=8))
            lg_s = big.tile([128, 128], F16, tag="lg")
            nc.sync.dma_start(lg_s[:], lg[:])

            # ---- x^T in 1024-col halves (2KB DMA lines), need-ordered ----
            xt = [xtp.tile([128, N], F16, tag="xt", name=f"xt{c}")
                  for c in range(8)]
            for c in range(8):
                nc.sync.dma_start(xt[c][:, 0:1024], xT[c * 128:(c + 1) * 128, 0:1024])
            nc.sync.dma_start(wq_s[:], wq[:].rearrange("p (c d) -> p c d", c=8))
            nc.sync.dma_start(wv_s[:], wv[:].rearrange("p (c d) -> p c d", c=8))
            # host-baked causal masks (per-half data): pos0 slots first
            mk_s = big.tile([128, NSLOT * 512], F16, tag="mk")
            nc.sync.dma_start(mk_s[:, 0:4096], mk[:, 0:4096])
            for c in range(8):
                nc.sync.dma_start(xt[c][:, 1024:2048],
                                  xT[c * 128:(c + 1) * 128, 1024:2048])
            nc.sync.dma_start(mk_s[:, 4096:8192], mk[:, 4096:8192])

            # ---- persistent intermediates ----
            kT = big.tile([128, N], F16, tag="kT")
            qT = big.tile([128, N // 2], F16, tag="qT")
            yT = big.tile([128, N], F16, tag="yT")
            sqn = big.tile([128, NBLK], F32, tag="sqn")
            ek_s = big.tile([128, NBLK], F32, tag="ek")
            mkg = big.tile([128, NSLOT * 512], F16, tag="mkg")
            vaug = big.tile([128, NBLK, 132], F16, tag="vaug")
            nc.vector.memset(vaug[:, :, 128:129], 1.0 / OMEGA_LANG)
            nc.vector.memset(vaug[:, :, 129:130], 1.0 / OMEGA_GRAV)

            def proj_group(dst, dcol, w_sb, g):
                cols = slice(g * 512, (g + 1) * 512)
                ps = pp.tile([128, 512], F32, tag="pp")
                for c in range(8):
                    nc.tensor.matmul(ps[:], w_sb[:, c, :], xt[c][:, cols],
                                     start=(c == 0), stop=(c == 7))
                nc.vector.tensor_copy(dst[:, dcol * 512:(dcol + 1) * 512], ps[:])

            def yt_group(g):
                cols = slice(g * 512, (g + 1) * 512)
                ps = pp.tile([128, 512], F32, tag="pp")
                nc.tensor.matmul(ps[:], lg_s[:], kT[:, cols])
                nc.vector.tensor_copy(yT[:, cols], ps[:])

            def sqn_quad(q4):
                # y chunks for 4 key blocks -> [128,4,128] psum; squared on
                # ACT ((y/16)^2 = y^2/256), then one DVE reduce -> sq/256.
                ps = pp.tile([128, 4, 128], F32, tag="pp")
                for i in range(4):
                    kb = q4 * 4 + i
                    nc.tensor.matmul(ps[:, i, :], kT[:, kb * 128:(kb + 1) * 128],
                                     lg_s[:])
                scr = small.tile([128, 4, 128], F32, tag="scr")
                nc.scalar.activation(scr[:], ps[:], AF.Square, scale=0.0625)
                nc.vector.tensor_reduce(sqn[:, q4 * 4:(q4 + 1) * 4], scr[:],
                                        mybir.AxisListType.X, OP.add)

            def vaug_chunk(kb):
                ps = pp.tile([128, 512], F32, tag="pp")
                for c in range(8):
                    nc.tensor.matmul(ps[:, 0:128], xt[c][:, kb * 128:(kb + 1) * 128],
                                     wv_s[:, c, :], start=(c == 0), stop=(c == 7))
                nc.vector.tensor_copy(vaug[:, kb, 0:128], ps[:, 0:128])

            # PE warmup woven into the first kT projection group: 3 dummy
            # matmuls per x chunk, each consuming only already-arrived data,
            # so the PE tracks DMA arrival and the HAM clock-gate opens
            # (4/8 -> 8/8) by the time the dense stream begins.
            def kt_g0_with_warmup():
                warm = pp.tile([128, 512], F32, tag="pp")
                ps = pp.tile([128, 512], F32, tag="pp")
                for c in range(8):
                    for i in range(3):
                        nc.tensor.matmul(warm[:], xt[c][:, 0:128],
                                         xt[c][:, 0:512],
                                         start=(c == 0 and i == 0),
                                         stop=(c == 7 and i == 2))
                    nc.tensor.matmul(ps[:], wk_s[:, c, :], xt[c][:, 0:512],
                                     start=(c == 0), stop=(c == 7))
                nc.vector.tensor_copy(kT[:, 0:512], ps[:])

            def score_pair(dst, lhs, slot, plist, qrhs, scale):
                # two 512-wide score matmuls (chunks plist[slot], plist[slot+1])
                # -> one [128,1024] psum tile, one biasless exp ACT.
                ca, cb = plist[slot], plist[slot + 1]
                ps = score.tile([128, 1024], F32, tag="sc")
                nc.tensor.matmul(ps[:, 0:512], lhs[:, ca * 128:(ca + 1) * 128], qrhs)
                nc.tensor.matmul(ps[:, 512:1024], lhs[:, cb * 128:(cb + 1) * 128],
                                 qrhs)
                nc.scalar.activation(dst[:, slot * 512:(slot + 2) * 512], ps[:],
                                     AF.Exp, scale=scale)

            # ================= pos0 prologue =================
            kt_g0_with_warmup()
            proj_group(kT, 1, wk_s, 1)
            proj_group(qT, 0, wq_s, 0)
            q0 = qT[:, 0:512]
            yq0 = yT[:, 0:512]

            alang = [None, None]
            agrav = [None, None]
            alang[0] = ap0.tile([128, NCH[0] * 512], F16, tag="al0", name="al0")
            agrav[0] = ap0.tile([128, NCH[0] * 512], F16, tag="ag0", name="ag0")

            # lang scores pos0 interleaved with remaining prologue PE work
            score_pair(alang[0], kT, 0, P0, q0, SC_LANG)
            yt_group(0)
            score_pair(alang[0], kT, 2, P0, q0, SC_LANG)
            yt_group(1)
            score_pair(alang[0], kT, 4, P0, q0, SC_LANG)
            sqn_quad(0)
            score_pair(alang[0], kT, 6, P0, q0, SC_LANG)
            sqn_quad(1)
            nc.scalar.activation(ek_s[:, 0:8], sqn[:, 0:8], AF.Exp, scale=-1.0)
            for s in range(8):
                mcols = slice(s * 512, (s + 1) * 512)
                nc.vector.tensor_scalar(mkg[:, mcols], mk_s[:, mcols],
                                        ek_s[:, P0[s]:P0[s] + 1], None, OP.mult)

            # grav scores pos0 interleaved with vaug chunks
            for p in range(4):
                score_pair(agrav[0], yT, 2 * p, P0, yq0, SC_GRAV)
                vaug_chunk(2 * p)
                vaug_chunk(2 * p + 1)

            # pos0 masking: lang mask-mult, grav ek-premultiplied mask-mult
            for s in range(8):
                acols = slice(s * 512, (s + 1) * 512)
                nc.vector.tensor_tensor(alang[0][:, acols], alang[0][:, acols],
                                        mk_s[:, acols], OP.mult)
                nc.vector.tensor_tensor(agrav[0][:, acols], agrav[0][:, acols],
                                        mkg[:, acols], OP.mult)

            # ================= pos1 prologue (fills PE while pos0 exp/mask run)
            proj_group(kT, 2, wk_s, 2)
            proj_group(kT, 3, wk_s, 3)
            proj_group(qT, 1, wq_s, 2)
            q1 = qT[:, 512:1024]
            yq1 = yT[:, 1024:1536]

            # pos1 lang scores interleaved with rest of pos1 prologue
            alang[1] = ap1.tile([128, NCH[1] * 512], F16, tag="al1", name="al1")
            agrav[1] = ap1.tile([128, NCH[1] * 512], F16, tag="ag1", name="ag1")
            score_pair(alang[1], kT, 0, P1, q1, SC_LANG)
            yt_group(2)
            score_pair(alang[1], kT, 2, P1, q1, SC_LANG)
            yt_group(3)
            score_pair(alang[1], kT, 4, P1, q1, SC_LANG)
            sqn_quad(2)
            score_pair(alang[1], kT, 6, P1, q1, SC_LANG)
            sqn_quad(3)
            score_pair(alang[1], kT, 8, P1, q1, SC_LANG)
            nc.scalar.activation(ek_s[:, 8:16], sqn[:, 8:16], AF.Exp, scale=-1.0)
            for s in range(8, 16):
                mcols = slice(s * 512, (s + 1) * 512)
                nc.vector.tensor_scalar(mkg[:, mcols], mk_s[:, mcols],
                                        ek_s[:, P1[s]:P1[s] + 1], None, OP.mult)
            score_pair(alang[1], kT, 10, P1, q1, SC_LANG)
            vaug_chunk(8)
            vaug_chunk(9)
            score_pair(alang[1], kT, 12, P1, q1, SC_LANG)
            vaug_chunk(10)
            vaug_chunk(11)
            score_pair(alang[1], kT, 14, P1, q1, SC_LANG)
            for kb in range(12, 16):
                vaug_chunk(kb)

            def attn_j(pos, j):
                # A^T @ v_aug for one 128-row query block; rowsums ride col 128.
                nkb = (5 + j) if pos == 0 else (13 + j)
                plist = P0 if pos == 0 else P1
                pol = av.tile([128, 132], F32, tag="av")
                pog = av.tile([128, 132], F32, tag="av")
                for kb in range(nkb):
                    nc.tensor.matmul(pol[:, 0:129],
                                     alang[pos][:, kb * 512 + j * 128:kb * 512 + (j + 1) * 128],
                                     vaug[:, plist[kb], 0:129],
                                     start=(kb == 0), stop=(kb == nkb - 1))
                for kb in range(nkb):
                    nc.tensor.matmul(pog[:, 0:130],
                                     agrav[pos][:, kb * 512 + j * 128:kb * 512 + (j + 1) * 128],
                                     vaug[:, plist[kb], 0:130],
                                     start=(kb == 0), stop=(kb == nkb - 1))
                rl = small.tile([128, 1], F32, tag="rl")
                rg = small.tile([128, 1], F32, tag="rg")
                nc.vector.reciprocal(rl[:], pol[:, 128:129])
                nc.vector.reciprocal(rg[:], pog[:, 129:130])
                ob = outp.tile([128, 128], F32, tag="ob")
                ob2 = outp.tile([128, 128], F32, tag="ob2")
                nc.vector.tensor_scalar(ob[:], pol[:, 0:128], rl[:], None, OP.mult)
                nc.vector.scalar_tensor_tensor(ob2[:], pog[:, 0:128], rg[:], ob[:],
                                               OP.mult, OP.add)
                r0 = pos * 512 + j * 128
                nc.sync.dma_start(out_d[r0:r0 + 128, :], ob2[:])

            # pos0 A@v interleaved with pos1 grav scores (keeps ACT fed).
            # pos1 masking rides along per-slot: loop slots 0-7 are fully
            # valid for both halves (grav needs only the ek factor;
            # P1[0..7] == chunks 0..7); slots 8-15 get mask multiplies.
            def post1(s):
                acols = slice(s * 512, (s + 1) * 512)
                if s < 8:
                    nc.vector.tensor_scalar(agrav[1][:, acols],
                                            agrav[1][:, acols],
                                            ek_s[:, s:s + 1], None, OP.mult)
                else:
                    nc.vector.tensor_tensor(alang[1][:, acols],
                                            alang[1][:, acols],
                                            mk_s[:, acols], OP.mult)
                    nc.vector.tensor_tensor(agrav[1][:, acols],
                                            agrav[1][:, acols],
                                            mkg[:, acols], OP.mult)

            score_pair(agrav[1], yT, 0, P1, yq1, SC_GRAV)
            post1(0); post1(1)
            attn_j(0, 0)
            score_pair(agrav[1], yT, 2, P1, yq1, SC_GRAV)
            post1(2); post1(3)
            score_pair(agrav[1], yT, 4, P1, yq1, SC_GRAV)
            post1(4); post1(5)
            attn_j(0, 1)
            score_pair(agrav[1], yT, 6, P1, yq1, SC_GRAV)
            post1(6); post1(7)
            score_pair(agrav[1], yT, 8, P1, yq1, SC_GRAV)
            post1(8); post1(9)
            attn_j(0, 2)
            score_pair(agrav[1], yT, 10, P1, yq1, SC_GRAV)
            post1(10); post1(11)
            score_pair(agrav[1], yT, 12, P1, yq1, SC_GRAV)
            post1(12); post1(13)
            attn_j(0, 3)
            score_pair(agrav[1], yT, 14, P1, yq1, SC_GRAV)
            post1(14); post1(15)

            # pos1 A@v + outputs (longest chain first, shortest last)
            for j in (3, 2, 1, 0):
                attn_j(1, j)

    nc.finalize()
    return nc


def _host_inputs(x, Wq, Wk, Wv, L_grav):
    """Build the 8 per-core input maps (permuted key layout per half)."""
    f16 = np.float16
    x = np.asarray(x, np.float32)
    Wq = np.asarray(Wq, np.float32)
    Wk = np.asarray(Wk, np.float32)
    Wv = np.asarray(Wv, np.float32)
    L = np.asarray(L_grav, np.float32)

    def warr(w):  # [1024,128] -> [128, 8*128] chunk-major for lhsT slices
        return np.ascontiguousarray(
            w.reshape(8, 128, 128).transpose(1, 0, 2).reshape(128, 8 * 128)
        ).astype(f16)

    wqa, wka, wva = warr(Wq), warr(Wk), warr(Wv)
    lga = L.astype(f16)

    def half_mask(h):
        """mk [128, 16*512] f16 masks; mask slot s covers loop slot s.

        mask[k_p, q_local] = (q_orig >= key_orig), fully data-driven over
        the permuted key order. pos0 -> slots 0-7 (chunks P0), pos1 ->
        slots 8-15 (chunks P1[8..15]).
        """
        groups = GROUPS[h]
        p = np.arange(128, dtype=np.float32)[:, None]
        q = np.arange(512, dtype=np.float32)[None, :]
        mkh = np.empty((128, NSLOT, 512), np.float32)
        for s in range(NSLOT):
            if s < 8:
                c, qg = P0[s], groups[0]
            else:
                c, qg = P1[s], groups[2]
            og = groups[c // 4]
            key_orig = og * 512 + (c % 4) * 128 + p   # [128,1]
            thr = key_orig - qg * 512
            mkh[:, s, :] = (q >= thr)
        return np.ascontiguousarray(mkh.reshape(128, NSLOT * 512)).astype(f16)

    mks = [half_mask(0), half_mask(1)]
    in_maps = []
    for core in range(8):
        b, h = core // 2, core % 2
        xTb = x[b].T.astype(f16)  # [1024, 2048]
        xp = np.concatenate([xTb[:, g * 512:(g + 1) * 512] for g in GROUPS[h]],
                            axis=1)
        in_maps.append({
            "xT": np.ascontiguousarray(xp),
            "wq": wqa, "wk": wka, "wv": wva, "lg": lga,
            "mk": mks[h],
        })
    return in_maps


def kernel(x, Wq, Wk, Wv, L_grav):
    import concourse.bass_utils as bass_utils

    if "nc" not in _CACHE:
        _CACHE["nc"] = _build_nc()
    nc = _CACHE["nc"]
    in_maps = _host_inputs(x, Wq, Wk, Wv, L_grav)

    trace = bool(os.environ.get("BASS_KERNEL_TRACE"))
    if trace:
        bass_utils.upload_artifacts = lambda tmpdir: f"file://{tmpdir}"
    res = bass_utils.run_bass_kernel_spmd(nc, in_maps, list(range(8)), trace=trace)
    if trace:
        _CACHE["exec_time_ns"] = res.exec_time_ns
        _CACHE["mean_exec_time_ns"] = res.mean_exec_time_ns

    out = np.empty((B, N, D_HEAD), np.float32)
    for core in range(8):
        b, h = core // 2, core % 2
        r = res.results[core]["out"]
        g0, g2 = GROUPS[h][0], GROUPS[h][2]
        out[b, g0 * 512:(g0 + 1) * 512] = r[0:512]
        out[b, g2 * 512:(g2 + 1) * 512] = r[512:1024]
    return out
